# revision 7
# baseline (speedup 1.0000x reference)
"""DynamicLinear (MoE routing) Trainium2 Bass kernel.

Math (per sample b):
    out[b] = sum_k attn[b,k] * (x[b] @ W[k].T + bias[k])
           = sum_k attn[b,k] * (x[b] @ W[k].T) + attn[b] @ bias

Sharding: 8 cores in a 2x4 grid over (batch, out_features).
Each core computes out[b_half, o_quarter] from x[b_half] (16 MiB fp32),
W[:, o_quarter, :] (16 MiB fp32) -- no cross-core communication.

Per-core pipeline (expert-outer so TensorE starts after only W[0] is
staged, not all four experts):
  1. gpsimd casting DMAs: x / W fp32 DRAM -> bf16 SBUF natural tiles.
  2. xbar DMA transposes (bf16 SBUF->SBUF): put the contraction dim on
     partitions: xT[t] [128,16,128], wT[k] [128,4(oi),16(ii),128].
  3. TensorE, for k in 4: for b_tile in 16: accumulate 16 matmul passes
     (K=128 contraction, N=512 moving) into one PSUM bank.
  4. ACT+DVE combine into per-b_tile SBUF accumulators:
     acc[t] = sum_k attn[:,k]*(bias[k] + psum_k), attn as per-partition
     scalar (b lives on the partition dim).
  5. DMA acc -> out after the last expert.
"""

import numpy as np

_B, _K, _IN, _OUT = 4096, 4, 2048, 2048
_GRID_B, _GRID_O = 2, 4
_BL = _B // _GRID_B      # 2048 batch rows per core
_OL = _OUT // _GRID_O    # 512 out cols per core
_NBT = _BL // 128        # 16 b tiles
_NIT = _IN // 128        # 16 contraction tiles
_NOT = _OL // 128        # 4 o tiles

_CACHE = {}
LAST_RESULTS = None


def _build_program():
    import concourse.bass as bass
    import concourse.tile as tile
    from concourse import bacc, mybir

    f32 = mybir.dt.float32
    bf16 = mybir.dt.bfloat16
    MULT = mybir.AluOpType.mult
    ADD = mybir.AluOpType.add
    COPY = mybir.ActivationFunctionType.Copy

    nc = bacc.Bacc("TRN2", target_bir_lowering=False, debug=False)
    x = nc.dram_tensor("x", [_BL, _IN], f32, kind="ExternalInput").ap()
    attn = nc.dram_tensor("attn", [_BL, _K], f32, kind="ExternalInput").ap()
    w = nc.dram_tensor("w", [_K, _OL, _IN], f32, kind="ExternalInput").ap()
    bias = nc.dram_tensor("bias", [_K, _OL], f32, kind="ExternalInput").ap()
    out = nc.dram_tensor("out", [_BL, _OL], f32, kind="ExternalOutput").ap()

    with tile.TileContext(nc) as tc:
        with (
            tc.tile_pool(name="wnat", bufs=6) as wnatp,
            tc.tile_pool(name="xnat", bufs=4) as xnatp,
            tc.tile_pool(name="wT", bufs=2) as wTp,
            tc.tile_pool(name="xT", bufs=_NBT) as xTp,
            tc.tile_pool(name="singles", bufs=1) as singles,
            tc.tile_pool(name="attn", bufs=_NBT) as attnp,
            tc.tile_pool(name="acc", bufs=_NBT) as accp,
            tc.tile_pool(name="psum", bufs=8, space="PSUM") as psump,
        ):
            # bias replicated across all 128 partitions: [128, K, OL]
            bias_rep = singles.tile([128, _K, _OL], f32)
            bias_bcast = bass.AP(
                tensor=bias.tensor,
                offset=bias.offset,
                ap=[[0, 128], bias.ap[0], bias.ap[1]],
            )
            nc.gpsimd.dma_start(out=bias_rep, in_=bias_bcast)

            # attn tiles, b on partitions: [128, K] per b_tile
            attn_sb = []
            for t in range(_NBT):
                a = attnp.tile([128, _K], f32, tag="attn", name=f"attn{t}")
                nc.sync.dma_start(out=a, in_=attn[t * 128:(t + 1) * 128, :])
                attn_sb.append(a)

            def stage_w(k):
                """cast W[k] to bf16 and transpose to [128,4,16,128]."""
                wt = wTp.tile([128, _NOT, _NIT, 128], bf16, tag="wT",
                              name=f"wT{k}")
                for oi in range(_NOT):
                    wn = wnatp.tile([128, _IN], bf16, tag="wnat",
                                    name=f"wnat{k}_{oi}")
                    nc.gpsimd.dma_start(
                        out=wn, in_=w[k, oi * 128:(oi + 1) * 128, :]
                    )
                    # wT[k][i_in, oi, ii, o_in] = W[k][oi*128+o_in, ii*128+i_in]
                    nc.sync.dma_start_transpose(wt[:, oi], wn)
                return wt

            def stage_x(t):
                xn = xnatp.tile([128, _IN], bf16, tag="xnat", name=f"xnat{t}")
                nc.gpsimd.dma_start(out=xn, in_=x[t * 128:(t + 1) * 128, :])
                xt = xTp.tile([128, _NIT, 128], bf16, tag="xT", name=f"xT{t}")
                nc.sync.dma_start_transpose(xt, xn)
                return xt

            wT = {0: stage_w(0)}
            xT = [stage_x(t) for t in range(_NBT)]
            acc = [None] * _NBT

            for k in range(_K):
                if k + 1 < _K:
                    wT[k + 1] = stage_w(k + 1)  # prefetch next expert
                for t in range(_NBT):
                    ps = psump.tile([128, _OL], f32, tag="ps",
                                    name=f"ps{k}_{t}")
                    for ii in range(_NIT):
                        nc.tensor.matmul(
                            ps, lhsT=xT[t][:, ii, :], rhs=wT[k][:, :, ii, :],
                            start=(ii == 0), stop=(ii == _NIT - 1),
                        )
                    a_t = attn_sb[t]
                    if k == 0:
                        # init acc with the full bias combination
                        at = accp.tile([128, _OL], f32, tag="acc",
                                       name=f"acc{t}")
                        acc[t] = at
                        nc.scalar.activation(
                            at, bias_rep[:, 0, :], COPY, scale=a_t[:, 0:1]
                        )
                        for kk in range(1, _K):
                            nc.vector.scalar_tensor_tensor(
                                out=at, in0=bias_rep[:, kk, :],
                                scalar=a_t[:, kk:kk + 1], in1=at,
                                op0=MULT, op1=ADD,
                            )
                    nc.vector.scalar_tensor_tensor(
                        out=acc[t], in0=ps, scalar=a_t[:, k:k + 1],
                        in1=acc[t], op0=MULT, op1=ADD,
                    )
                    if k == _K - 1:
                        nc.sync.dma_start(
                            out=out[t * 128:(t + 1) * 128, :], in_=acc[t]
                        )
                del wT[k]

    nc.compile()
    return nc


def _get_program():
    if "nc" not in _CACHE:
        _CACHE["nc"] = _build_program()
    return _CACHE["nc"]


def _ensure_axon_hooks_importable():
    """bass_utils' trace branch imports antenv.axon_hooks, which the
    trimmed agent image may lack; stub it (hook=None) so a stray
    BASS_TRACE=1 degrades to an untraced run instead of crashing."""
    import sys
    import types

    try:
        import antenv.axon_hooks  # noqa: F401
        return
    except ImportError:
        pass
    mod = types.ModuleType("antenv.axon_hooks")
    mod._hook = None
    mod.get_axon_ntff_profile_hook = lambda: mod._hook

    def _set(h):
        mod._hook = h

    mod.set_axon_ntff_profile_hook = _set
    sys.modules["antenv.axon_hooks"] = mod
    try:
        import antenv
        antenv.axon_hooks = mod
    except ImportError:
        pass


def kernel(**inputs):
    global LAST_RESULTS
    from concourse.bass_utils import run_bass_kernel_spmd

    _ensure_axon_hooks_importable()

    x = np.ascontiguousarray(inputs["x"], dtype=np.float32)
    attn = np.ascontiguousarray(inputs["softmax_attention"], dtype=np.float32)
    w = np.ascontiguousarray(inputs["weight"], dtype=np.float32)
    b = np.ascontiguousarray(inputs["bias"], dtype=np.float32)

    nc = _get_program()
    in_maps = []
    for c in range(8):
        gb, go = divmod(c, _GRID_O)
        in_maps.append({
            "x": np.ascontiguousarray(x[gb * _BL:(gb + 1) * _BL]),
            "attn": np.ascontiguousarray(attn[gb * _BL:(gb + 1) * _BL]),
            "w": np.ascontiguousarray(w[:, go * _OL:(go + 1) * _OL, :]),
            "bias": np.ascontiguousarray(b[:, go * _OL:(go + 1) * _OL]),
        })

    res = run_bass_kernel_spmd(nc, in_maps, list(range(8)))
    LAST_RESULTS = res

    full = np.empty((_B, _OUT), dtype=np.float32)
    for c in range(8):
        gb, go = divmod(c, _GRID_O)
        full[gb * _BL:(gb + 1) * _BL, go * _OL:(go + 1) * _OL] = \
            res.results[c]["out"]
    return full


# revision 9
# speedup vs baseline: 1.2050x; 1.2050x over previous
"""DynamicLinear (MoE routing) Trainium2 Bass kernel.

Math (per sample b):
    out[b] = sum_k attn[b,k] * (x[b] @ W[k].T + bias[k])
           = sum_k attn[b,k] * (x[b] @ W[k].T) + attn[b] @ bias

Sharding: 8 cores in a 2x4 grid over (batch, out_features).
Each core computes out[b_half, o_quarter] from x[b_half] (16 MiB fp32),
W[:, o_quarter, :] (16 MiB fp32) -- no cross-core communication.

Per-core pipeline (expert-outer so TensorE starts once W[0] is staged):
  1. gpsimd casting DMAs: x / W fp32 -> bf16 DRAM staging.
  2. xbar DMA transposes (bf16, DRAM -> SBUF, one instruction per
     expert / x-group): wT[k] [128,16(ii),512(o)], xT[g] [128,16,512(b)].
     W transposes issue on the sync ring, x transposes on the scalar
     ring -- the ucode transpose blocks its issuing engine, so the two
     streams must not share one ring.
  3. TensorE, for k in 4: for b_tile in 16: accumulate 16 matmul passes
     (K=128 contraction, N=512 contiguous moving) into one PSUM bank.
  4. ACT+DVE combine into per-b_tile SBUF accumulators:
     acc[t] = sum_k attn[:,k]*(bias[k] + psum_k), attn as per-partition
     scalar (b lives on the partition dim).
  5. DMA acc -> out after the last expert.
"""

import numpy as np

_B, _K, _IN, _OUT = 4096, 4, 2048, 2048
_GRID_B, _GRID_O = 2, 4
_BL = _B // _GRID_B      # 2048 batch rows per core
_OL = _OUT // _GRID_O    # 512 out cols per core
_NBT = _BL // 128        # 16 b tiles
_NIT = _IN // 128        # 16 contraction tiles
_NOT = _OL // 128        # 4 o row-tiles of W
_XG = 512                # batch rows per x-transpose group
_NG = _BL // _XG         # 4 groups

_CACHE = {}
LAST_RESULTS = None


def _build_program():
    import concourse.bass as bass
    import concourse.tile as tile
    from concourse import bacc, mybir

    f32 = mybir.dt.float32
    bf16 = mybir.dt.bfloat16
    MULT = mybir.AluOpType.mult
    ADD = mybir.AluOpType.add
    COPY = mybir.ActivationFunctionType.Copy

    nc = bacc.Bacc("TRN2", target_bir_lowering=False, debug=False)
    x = nc.dram_tensor("x", [_BL, _IN], f32, kind="ExternalInput").ap()
    attn = nc.dram_tensor("attn", [_BL, _K], f32, kind="ExternalInput").ap()
    w = nc.dram_tensor("w", [_K, _OL, _IN], f32, kind="ExternalInput").ap()
    bias = nc.dram_tensor("bias", [_K, _OL], f32, kind="ExternalInput").ap()
    out = nc.dram_tensor("out", [_BL, _OL], f32, kind="ExternalOutput").ap()

    with tile.TileContext(nc) as tc:
        with (
            tc.tile_pool(name="dram", bufs=1, space="DRAM") as dram,
            tc.tile_pool(name="wT", bufs=2) as wTp,
            tc.tile_pool(name="xT", bufs=_NG) as xTp,
            tc.tile_pool(name="singles", bufs=1) as singles,
            tc.tile_pool(name="acc", bufs=_NBT) as accp,
            tc.tile_pool(name="psum", bufs=8, space="PSUM") as psump,
        ):
            wbf = dram.tile([_K, _OL, _IN], bf16)
            xbf = dram.tile([_BL, _IN], bf16)

            def cast_w(k):
                for oi in range(_NOT):
                    nc.gpsimd.dma_start(
                        out=wbf[k, oi * 128:(oi + 1) * 128, :],
                        in_=w[k, oi * 128:(oi + 1) * 128, :],
                    )

            def cast_x(g):
                for sub in range(2):
                    r0 = g * _XG + sub * (_XG // 2)
                    r1 = r0 + _XG // 2
                    nc.gpsimd.dma_start(out=xbf[r0:r1, :], in_=x[r0:r1, :])

            def trans_w(k):
                # wT[k][i_in, ii, o] = W[k][o, ii*128 + i_in]
                wt = wTp.tile([128, _NIT, _OL], bf16, tag="wT", name=f"wT{k}")
                nc.sync.dma_start_transpose(wt, wbf[k])
                return wt

            def trans_x(g):
                # xT[g][i_in, ii, b] = x[g*XG + b, ii*128 + i_in]
                xt = xTp.tile([128, _NIT, _XG], bf16, tag="xT", name=f"xT{g}")
                nc.scalar.dma_start_transpose(
                    xt, xbf[g * _XG:(g + 1) * _XG, :]
                )
                return xt

            # attn for all b_tiles in one strided load, b on partitions:
            # attn_sb[p, t, k] = attn[t*128 + p, k]   (scalar ring)
            attn_sb = singles.tile([128, _NBT, _K], f32)
            attn_src = bass.AP(
                tensor=attn.tensor,
                offset=attn.offset,
                ap=[[_K, 128], [128 * _K, _NBT], [1, _K]],
            )
            nc.scalar.dma_start(out=attn_sb, in_=attn_src)

            # staging in need-order; the two transpose rings drain FIFO
            cast_w(0)
            cast_x(0)
            wT = {0: trans_w(0)}
            xT = {0: trans_x(0)}
            bias_rep = singles.tile([128, _K, _OL], f32)
            nc.gpsimd.dma_start(
                out=bias_rep,
                in_=bass.AP(
                    tensor=bias.tensor,
                    offset=bias.offset,
                    ap=[[0, 128], bias.ap[0], bias.ap[1]],
                ),
            )
            cast_w(1)
            cast_x(1)
            wT[1] = trans_w(1)
            xT[1] = trans_x(1)
            cast_w(2)
            cast_x(2)
            cast_w(3)
            cast_x(3)
            wT[2] = trans_w(2)
            xT[2] = trans_x(2)
            wT[3] = trans_w(3)
            xT[3] = trans_x(3)

            acc = [None] * _NBT
            for k in range(_K):
                for t in range(_NBT):
                    g, bq = divmod(t, _XG // 128)
                    ps = psump.tile([128, _OL], f32, tag="ps",
                                    name=f"ps{k}_{t}")
                    for ii in range(_NIT):
                        nc.tensor.matmul(
                            ps,
                            lhsT=xT[g][:, ii, bq * 128:(bq + 1) * 128],
                            rhs=wT[k][:, ii, :],
                            start=(ii == 0), stop=(ii == _NIT - 1),
                        )
                    a_sc = attn_sb[:, t, :]
                    if k == 0:
                        # init acc with the full bias combination
                        at = accp.tile([128, _OL], f32, tag="acc",
                                       name=f"acc{t}")
                        acc[t] = at
                        nc.scalar.activation(
                            at, bias_rep[:, 0, :], COPY, scale=a_sc[:, 0:1]
                        )
                        for kk in range(1, _K):
                            nc.vector.scalar_tensor_tensor(
                                out=at, in0=bias_rep[:, kk, :],
                                scalar=a_sc[:, kk:kk + 1], in1=at,
                                op0=MULT, op1=ADD,
                            )
                    nc.vector.scalar_tensor_tensor(
                        out=acc[t], in0=ps, scalar=a_sc[:, k:k + 1],
                        in1=acc[t], op0=MULT, op1=ADD,
                    )
                    if k == _K - 1:
                        nc.sync.dma_start(
                            out=out[t * 128:(t + 1) * 128, :], in_=acc[t]
                        )

    nc.compile()
    return nc


def _get_program():
    if "nc" not in _CACHE:
        _CACHE["nc"] = _build_program()
    return _CACHE["nc"]


def _ensure_axon_hooks_importable():
    """bass_utils' trace branch imports antenv.axon_hooks, which the
    trimmed agent image may lack; stub it (hook=None) so a stray
    BASS_TRACE=1 degrades to an untraced run instead of crashing."""
    import sys
    import types

    try:
        import antenv.axon_hooks  # noqa: F401
        return
    except ImportError:
        pass
    mod = types.ModuleType("antenv.axon_hooks")
    mod._hook = None
    mod.get_axon_ntff_profile_hook = lambda: mod._hook

    def _set(h):
        mod._hook = h

    mod.set_axon_ntff_profile_hook = _set
    sys.modules["antenv.axon_hooks"] = mod
    try:
        import antenv
        antenv.axon_hooks = mod
    except ImportError:
        pass


def kernel(**inputs):
    global LAST_RESULTS
    from concourse.bass_utils import run_bass_kernel_spmd

    _ensure_axon_hooks_importable()

    x = np.ascontiguousarray(inputs["x"], dtype=np.float32)
    attn = np.ascontiguousarray(inputs["softmax_attention"], dtype=np.float32)
    w = np.ascontiguousarray(inputs["weight"], dtype=np.float32)
    b = np.ascontiguousarray(inputs["bias"], dtype=np.float32)

    nc = _get_program()
    in_maps = []
    for c in range(8):
        gb, go = divmod(c, _GRID_O)
        in_maps.append({
            "x": np.ascontiguousarray(x[gb * _BL:(gb + 1) * _BL]),
            "attn": np.ascontiguousarray(attn[gb * _BL:(gb + 1) * _BL]),
            "w": np.ascontiguousarray(w[:, go * _OL:(go + 1) * _OL, :]),
            "bias": np.ascontiguousarray(b[:, go * _OL:(go + 1) * _OL]),
        })

    res = run_bass_kernel_spmd(nc, in_maps, list(range(8)))
    LAST_RESULTS = res

    full = np.empty((_B, _OUT), dtype=np.float32)
    for c in range(8):
        gb, go = divmod(c, _GRID_O)
        full[gb * _BL:(gb + 1) * _BL, go * _OL:(go + 1) * _OL] = \
            res.results[c]["out"]
    return full
